# revision 37
# baseline (speedup 1.0000x reference)
"""AFM (attentional factorization machine) kernel for 8 TRN2 NeuronCores.

Math: reference applies softmax over a size-1 axis => att == 1.0 exactly,
so the attention MLP (W_w, W_b, h_w, h_b) is dead code and

    out[b] = sum_{i<j} (x[b,i,:] * x[b,j,:]) @ p_w
           = 0.5 * ((sum_f x[b,f,:])^2 - sum_f x[b,f,:]^2) @ p_w

Sharding: pure data parallel on batch (2048 -> 8 x 256); p_w replicated
(packed into the tail of each core's input block).

Per-core kernel (layout [128 partitions = batch, free = (c=2 groups, f=32, e=64)]):
  - one fp16 DMA per core (x shard + replicated 0.5*p_w columns)
  - ScalarE (ACT): squares the x block
  - VectorE (DVE): two 5-level halving trees over f (2x_1p fp16 mode),
    then a small fp16 tail: S^2 - Q, * (0.5*p_w), f32 reduce over e.
"""

import numpy as np

N_CORES = 8
B = 2048
F = 32
E = 64
B_LOC = B // N_CORES        # 256
C = B_LOC // 128            # 2 batch groups per core
FE = F * E                  # 2048
XCOLS = C * FE + C * E      # 4352: x block + pw block

_BUILD_CACHE: dict = {}


def _build(
    dt_x_name: str = "float16",
    use_act: bool = True,
    repeat: int = 1,
    loopable: bool | str = False,
    bufs: int | None = None,
    probe: str | None = None,
    gq1: bool = False,
):
    import concourse.bass as bass
    import concourse.mybir as mybir
    import concourse.tile as tile

    dt_x = getattr(mybir.dt, dt_x_name)
    f32 = mybir.dt.float32

    import types

    from concourse.tile_scheduler import N_PROCS
    from concourse.vector_clock import ScopedClock, VectorClock

    nc = bass.Bass()
    x_p = nc.declare_dram_parameter("xin", [128, XCOLS], dt_x, isOutput=False)
    out_p = nc.declare_dram_parameter("out", [128, C], f32, isOutput=True)

    def _split_drain_and_barrier(self, tick_clock, wait_clock):
        # Walrus rejects >~2 sync-wait commands on one instruction; stock
        # Tile attaches every outstanding proc's sem to the single final
        # drain. Pre-drain once per proc (1 wait each); add_sem_waits'
        # per-engine elision then leaves the real drain with no new waits.
        snc = self.nc
        gc = tick_clock.global_clock
        for p in range(N_PROCS):
            if gc[p] > 0:
                vc = VectorClock()
                vc.require_at_least(p, gc[p])
                d = snc.sync.drain()
                wait_clock.add_sem_waits(d.ins, ScopedClock({None: vc}))
        # The pre-drains above already blocked SP on every outstanding sem;
        # SP program order makes further waits on the final drain redundant.
        snc.sync.drain()
        snc.all_engine_barrier()
        assert self.sems is not None
        popped = snc._tile_sem_poison_stack.pop()
        assert popped is self._sem_poison
        snc.clear_and_free_semaphores(list(self.sems.allocated().values()))
        snc.all_engine_barrier()

    def _make_split_excess_waits(tc):
        # Walrus enforces <=1 sync wait per instruction (2 for
        # EventSemaphore); Tile's wait assignment does not. Before commit,
        # hoist excess waits onto same-engine NoOps inserted just ahead.
        orig_lower = tc._lower_ordered_insts

        def wrapper(ordered):
            fixed = {}
            for bb_name, insts in ordered.items():
                new_list = []
                for inst in insts:
                    si = inst.sync_info
                    cap = 2 if isinstance(inst, mybir.InstEventSemaphore) else 1
                    if si is not None and si.on_wait and len(si.on_wait) > cap:
                        waits = list(si.on_wait)
                        keep, excess = waits[-cap:], waits[:-cap]
                        for w in excess:
                            nop = mybir.InstNoOp(
                                name=f"I-{nc.next_id()}", ins=[], outs=[]
                            )
                            nop.engine = inst.engine
                            nop.sync_info = mybir.SyncInfo(
                                on_wait=[w], on_update=[]
                            )
                            for attr in (
                                "bass_scheduled_proc",
                                "bass_scheduled_tick",
                                "bass_scheduled_scope",
                                "bass_priority",
                            ):
                                try:
                                    setattr(nop, attr, getattr(inst, attr))
                                except Exception:
                                    pass
                            new_list.append(nop)
                        inst.sync_info = mybir.SyncInfo(
                            on_wait=keep, on_update=list(si.on_update or [])
                        )
                    new_list.append(inst)
                fixed[bb_name] = new_list
            return orig_lower(fixed)

        return wrapper

    tc_ctx = tile.TileContext(nc)
    tc_ctx._drain_and_barrier = types.MethodType(_split_drain_and_barrier, tc_ctx)
    tc_ctx._lower_ordered_insts = _make_split_excess_waits(tc_ctx)

    if bufs is None:
        bufs = 3 if repeat > 1 else 1
    R_HALF = 0.7071067811865476  # ACT free-affine scale: Square(r*u) = u^2/2

    with tc_ctx as tc:
        with tc.tile_pool(name="main", bufs=bufs) as pool:
            for it in range(repeat):
                X = pool.tile([128, XCOLS], dt_x, tag="X")
                nc.sync.dma_start(out=X[:], in_=x_p[:])
                PW = X[:, C * FE : XCOLS]                  # [128, C*E] fp16

                def view(ap, f):
                    return ap.rearrange("p (c f e) -> p c f e", c=C, f=f, e=E)

                def tree_levels(src_ap, f0, tag, first_eng=None):
                    """Halving adds over f (f0 -> 1); optionally the first
                    level on another engine (gpsimd) to offload DVE."""
                    cur, f = src_ap, f0
                    while f > 1:
                        h = f // 2
                        nxt = pool.tile([128, C * h * E], dt_x, tag=f"{tag}{h}")
                        cv, nv = view(cur, f), view(nxt[:], h)
                        eng = first_eng if (f == f0 and first_eng) else nc.vector
                        eng.tensor_add(nv, cv[:, :, 0:h, :], cv[:, :, h:f, :])
                        cur, f = nxt[:], h
                    return cur

                Xv = view(X[:, 0 : C * FE], F)
                if probe == "dmaonly":
                    res = pool.tile([128, C], f32, tag="res")
                    nc.vector.tensor_reduce(
                        res[:],
                        X[:, 0 : C * E].rearrange("p (c e) -> p c e", c=C, e=E),
                        axis=mybir.AxisListType.X,
                        op=mybir.AluOpType.add,
                    )
                    if it == repeat - 1:
                        nc.sync.dma_start(out=out_p[:], in_=res[:])
                    continue
                if loopable == "cat":
                    # Concatenated trees: s1 and q1 written into one tile so
                    # levels 2..5 process both with single ops.
                    if probe == "nosq":
                        SQ = X  # timing probe: skip the ACT square
                    else:
                        SQ = pool.tile([128, C * FE], dt_x, tag="SQ")
                        nc.scalar.activation(
                            SQ[:], X[:, 0 : C * FE],
                            mybir.ActivationFunctionType.Square,
                        )
                    SQv = view(SQ[:, 0 : C * FE], F)
                    H = F // 2
                    seg = C * H * E                       # 2048
                    L = pool.tile([128, 2 * seg], dt_x, tag="L1")
                    nc.vector.tensor_add(
                        view(L[:, 0:seg], H), Xv[:, :, 0:H, :], Xv[:, :, H:F, :]
                    )
                    nc.vector.tensor_add(
                        view(L[:, seg : 2 * seg], H),
                        SQv[:, :, 0:H, :], SQv[:, :, H:F, :],
                    )
                    cur, f = L, H
                    while f > 1:
                        h = f // 2
                        nxt = pool.tile([128, 2 * C * h * E], dt_x, tag=f"L{h}")
                        cv = cur[:].rearrange(
                            "p (g c f e) -> p g c f e", g=2, c=C, f=f, e=E
                        )
                        nv = nxt[:].rearrange(
                            "p (g c f e) -> p g c f e", g=2, c=C, f=h, e=E
                        )
                        nc.vector.tensor_add(
                            nv, cv[:, :, :, 0:h, :], cv[:, :, :, h:f, :]
                        )
                        cur, f = nxt, h
                    S = cur[:, 0 : C * E]
                    Q = cur[:, C * E : 2 * C * E]
                elif loopable:
                    # u/v form: X is read only by DVE; ACT reads DVE-made
                    # tiles. u=x_lo+x_hi, v=x_lo-x_hi; (u^2+v^2)/2 = q1.
                    H = F // 2
                    s1 = pool.tile([128, C * H * E], dt_x, tag="s1")
                    v1 = pool.tile([128, C * H * E], dt_x, tag="v1")
                    nc.vector.tensor_add(view(s1[:], H), Xv[:, :, 0:H, :], Xv[:, :, H:F, :])
                    nc.vector.tensor_sub(view(v1[:], H), Xv[:, :, 0:H, :], Xv[:, :, H:F, :])
                    qa = pool.tile([128, C * H * E], dt_x, tag="qa")
                    qb = pool.tile([128, C * H * E], dt_x, tag="qb")
                    nc.scalar.activation(
                        qa[:], s1[:], mybir.ActivationFunctionType.Square, scale=R_HALF
                    )
                    nc.scalar.activation(
                        qb[:], v1[:], mybir.ActivationFunctionType.Square, scale=R_HALF
                    )
                    q1 = pool.tile([128, C * H * E], dt_x, tag="q1")
                    nc.vector.tensor_add(q1[:], qa[:], qb[:])
                    S = tree_levels(s1[:], H, "s")
                    Q = tree_levels(q1[:], H, "q")
                else:
                    SQ = pool.tile([128, C * FE], dt_x, tag="SQ")
                    for c in range(C):
                        if use_act:
                            nc.scalar.activation(
                                SQ[:, c * FE : (c + 1) * FE],
                                X[:, c * FE : (c + 1) * FE],
                                mybir.ActivationFunctionType.Square,
                            )
                        else:
                            nc.vector.tensor_mul(
                                SQ[:, c * FE : (c + 1) * FE],
                                X[:, c * FE : (c + 1) * FE],
                                X[:, c * FE : (c + 1) * FE],
                            )
                    S = tree_levels(X[:, 0 : C * FE], F, "s")
                    Q = tree_levels(
                        SQ[:], F, "q", first_eng=nc.gpsimd if gq1 else None
                    )

                # fp16 tail: res[p, c] = sum_e (S^2 - Q) * (0.5 * p_w)
                T = pool.tile([128, C * E], dt_x, tag="T")
                V = pool.tile([128, C * E], dt_x, tag="V")
                W = pool.tile([128, C * E], dt_x, tag="W")
                res = pool.tile([128, C], f32, tag="res")
                nc.vector.tensor_mul(T[:], S, S)
                nc.vector.tensor_sub(V[:], T[:], Q)
                nc.vector.tensor_mul(W[:], V[:], PW)
                nc.vector.tensor_reduce(
                    res[:],
                    W[:].rearrange("p (c e) -> p c e", c=C, e=E),
                    axis=mybir.AxisListType.X,
                    op=mybir.AluOpType.add,
                )
                if it == repeat - 1:
                    nc.sync.dma_start(out=out_p[:], in_=res[:])

    return nc


USE_ACT = True


def _build_fold(
    e_plus: int,
    dt_x_name: str = "float16",
    repeat: int = 1,
    bufs: int | None = None,
    act_cols: int | None = None,
):
    """pw-folded variant.

    Host pre-scales x by w[e] = sqrt(0.5*|p_w[e]|) and permutes e-columns so
    the pw>=0 group (e_plus columns) comes first. Then with x' the scaled
    input, S' = sum_f x':

        out[b] = sum_{e in +} (S'^2 + ... ) :
        out[b] = [sum_+ S'^2 - sum_- S'^2] - [sum_+ x'^2 - sum_- x'^2]

    The second bracket (Q-dot) is computed entirely on ScalarE via
    Square+accum_out; the first on DVE (tree + 2 chained TTRs per group).
    `act_cols` (multiple of 16, <=64) limits how many e-columns ACT handles
    per c; the rest go to DVE as chained TTRs (load balance).
    """
    import types

    import concourse.bass as bass
    import concourse.mybir as mybir
    import concourse.tile as tile
    from concourse.tile_scheduler import N_PROCS
    from concourse.vector_clock import ScopedClock, VectorClock

    dt_x = getattr(mybir.dt, dt_x_name)
    f32 = mybir.dt.float32
    XC = C * FE

    nc = bass.Bass()
    x_p = nc.declare_dram_parameter("xin", [128, XC], dt_x, isOutput=False)
    out_p = nc.declare_dram_parameter("out", [128, C], f32, isOutput=True)

    def _split_drain_and_barrier(self, tick_clock, wait_clock):
        snc = self.nc
        gc = tick_clock.global_clock
        for p in range(N_PROCS):
            if gc[p] > 0:
                vc = VectorClock()
                vc.require_at_least(p, gc[p])
                d = snc.sync.drain()
                wait_clock.add_sem_waits(d.ins, ScopedClock({None: vc}))
        snc.sync.drain()
        snc.all_engine_barrier()
        assert self.sems is not None
        popped = snc._tile_sem_poison_stack.pop()
        assert popped is self._sem_poison
        snc.clear_and_free_semaphores(list(self.sems.allocated().values()))
        snc.all_engine_barrier()

    def _make_split_excess_waits(tc):
        orig_lower = tc._lower_ordered_insts

        def wrapper(ordered):
            fixed = {}
            for bb_name, insts in ordered.items():
                new_list = []
                for inst in insts:
                    si = inst.sync_info
                    cap = 2 if isinstance(inst, mybir.InstEventSemaphore) else 1
                    if si is not None and si.on_wait and len(si.on_wait) > cap:
                        waits = list(si.on_wait)
                        keep, excess = waits[-cap:], waits[:-cap]
                        for w in excess:
                            nop = mybir.InstNoOp(
                                name=f"I-{nc.next_id()}", ins=[], outs=[]
                            )
                            nop.engine = inst.engine
                            nop.sync_info = mybir.SyncInfo(on_wait=[w], on_update=[])
                            for attr in (
                                "bass_scheduled_proc",
                                "bass_scheduled_tick",
                                "bass_scheduled_scope",
                                "bass_priority",
                            ):
                                try:
                                    setattr(nop, attr, getattr(inst, attr))
                                except Exception:
                                    pass
                            new_list.append(nop)
                        inst.sync_info = mybir.SyncInfo(
                            on_wait=keep, on_update=list(si.on_update or [])
                        )
                    new_list.append(inst)
                fixed[bb_name] = new_list
            return orig_lower(fixed)

        return wrapper

    if bufs is None:
        bufs = 3 if repeat > 1 else 1
    if act_cols is None:
        act_cols = E

    tc_ctx = tile.TileContext(nc)
    tc_ctx._drain_and_barrier = types.MethodType(_split_drain_and_barrier, tc_ctx)
    tc_ctx._lower_ordered_insts = _make_split_excess_waits(tc_ctx)

    AL = mybir.AluOpType

    with tc_ctx as tc:
        with tc.tile_pool(name="main", bufs=bufs) as pool:
            for it in range(repeat):
                X = pool.tile([128, XC], dt_x, tag="X")
                # split by batch group so group-0 compute overlaps group-1 DMA
                for c in range(C):
                    nc.sync.dma_start(
                        out=X[:, c * FE : (c + 1) * FE],
                        in_=x_p[:, c * FE : (c + 1) * FE],
                    )

                JKA = pool.tile([128, FE], dt_x, tag="JKA")     # ACT junk out
                JKD = pool.tile([128, FE], dt_x, tag="JKD")     # DVE Q junk out

                # Contributions land as [128, C] strips in POS/NEG tiles:
                #   POS: +S'^2 pos-group, +Q neg-group
                #   NEG: +S'^2 neg-group, +Q pos-group
                # res = reduce(POS) - reduce(NEG).
                pos_strips: list = []
                neg_strips: list = []
                POS = pool.tile([128, 3 * C], f32, tag="POS")
                NEG = pool.tile([128, 3 * C], f32, tag="NEG")

                def strip(which):
                    lst, t = (
                        (pos_strips, POS) if which == "pos" else (neg_strips, NEG)
                    )
                    i = len(lst)
                    ap = t[:, i * C : (i + 1) * C]
                    lst.append(ap)
                    return ap

                # --- Q-dot: per (c, sign group): first act_cols columns on
                # ScalarE (Square+accum_out), remainder on DVE (mult+reduce).
                # One strip per (group, engine); column c written per c.
                e_neg = E - e_plus
                groups = []  # (g0, g1, which)
                groups.append((0, e_plus, "neg"))      # +Qpos contributes -
                if e_neg > 0:
                    groups.append((e_plus, E, "pos"))  # +Qneg contributes +
                gstrips = {}
                for gi, (g0, g1, which) in enumerate(groups):
                    na = min(act_cols, g1 - g0)
                    gstrips[gi] = (
                        strip(which) if na > 0 else None,
                        strip(which) if g0 + na < g1 else None,
                        na,
                    )
                for c in range(C):
                    xv = X[:, c * FE : (c + 1) * FE].rearrange(
                        "p (f e) -> p f e", f=F, e=E
                    )
                    for gi, (g0, g1, which) in enumerate(groups):
                        st_a, st_d, na = gstrips[gi]
                        if st_a is not None:
                            nc.scalar.activation(
                                JKA[:, 0 : F * na],
                                xv[:, :, g0 : g0 + na],
                                mybir.ActivationFunctionType.Square,
                                accum_out=st_a[:, c : c + 1],
                            )
                        if st_d is not None:
                            nd = g1 - (g0 + na)
                            sqr = JKD[:, 0 : F * nd]
                            nc.vector.tensor_mul(
                                sqr.rearrange("p (f e) -> p f e", f=F, e=nd),
                                xv[:, :, g0 + na : g1],
                                xv[:, :, g0 + na : g1],
                            )
                            nc.vector.tensor_reduce(
                                st_d[:, c : c + 1],
                                sqr.rearrange("p (f e) -> p f e", f=F, e=nd),
                                axis=mybir.AxisListType.XY,
                                op=AL.add,
                            )

                # --- S' tree on DVE: level 1 split per c (starts right
                # after that group's DMA), levels 2..5 c-joint.
                def view(ap, f):
                    return ap.rearrange("p (c f e) -> p c f e", c=C, f=f, e=E)

                H1 = F // 2
                s1 = pool.tile([128, C * H1 * E], dt_x, tag="s1")
                for c in range(C):
                    xc = X[:, c * FE : (c + 1) * FE].rearrange(
                        "p (f e) -> p f e", f=F, e=E
                    )
                    s1c = s1[:, c * H1 * E : (c + 1) * H1 * E].rearrange(
                        "p (f e) -> p f e", f=H1, e=E
                    )
                    nc.vector.tensor_add(s1c, xc[:, 0:H1, :], xc[:, H1:F, :])
                cur, f = s1[:], H1
                while f > 1:
                    h = f // 2
                    nxt = pool.tile([128, C * h * E], dt_x, tag=f"s{h}")
                    nc.vector.tensor_add(
                        view(nxt[:], h),
                        view(cur, f)[:, :, 0:h, :],
                        view(cur, f)[:, :, h:f, :],
                    )
                    cur, f = nxt[:], h
                S = cur  # [128, C*E] = sqrt(.5|pw|)*S, e-permuted

                # --- S-dot: T = S'*S' (f32), then per-sign-group reduces.
                T = pool.tile([128, C * E], f32, tag="T")
                nc.vector.tensor_mul(T[:], S, S)
                Tv = T[:].rearrange("p (c e) -> p c e", c=C, e=E)
                if e_plus > 0:
                    sp = strip("pos")
                    nc.vector.tensor_reduce(
                        sp, Tv[:, :, 0:e_plus], axis=mybir.AxisListType.X,
                        op=AL.add,
                    )
                if e_neg > 0:
                    sn = strip("neg")
                    nc.vector.tensor_reduce(
                        sn, Tv[:, :, e_plus:E], axis=mybir.AxisListType.X,
                        op=AL.add,
                    )

                # --- combine: res = rowsum(POS) - rowsum(NEG)
                res = pool.tile([128, C], f32, tag="res")
                RP = pool.tile([128, C], f32, tag="RP")
                RN = pool.tile([128, C], f32, tag="RN")
                np_, nn_ = len(pos_strips), len(neg_strips)

                def rowsum(dst, t, n):
                    if n == 0:
                        nc.vector.memset(dst, 0.0)
                        return
                    v = t[:, 0 : n * C].rearrange("p (r c) -> p c r", r=n, c=C)
                    nc.vector.tensor_reduce(
                        dst, v, axis=mybir.AxisListType.X, op=AL.add
                    )

                rowsum(RP[:], POS, np_)
                rowsum(RN[:], NEG, nn_)
                nc.vector.tensor_sub(res[:], RP[:], RN[:])
                if it == repeat - 1:
                    nc.sync.dma_start(out=out_p[:], in_=res[:])

    return nc


def _pack_inputs_fold(x, p_w, np_dt):
    """Per-core [128, C*FE] blocks of x scaled by sqrt(.5|pw|), e-sorted."""
    pw = p_w.reshape(E).astype(np.float64)
    perm = np.argsort(pw < 0, kind="stable")       # pw>=0 first
    e_plus = int((pw >= 0).sum())
    w = np.sqrt(0.5 * np.abs(pw[perm]))
    xs = x.reshape(B, F, E)[:, :, perm] * w[None, None, :]
    xs = xs.reshape(B, FE).astype(np_dt)
    blocks = []
    for i in range(N_CORES):
        sh = xs[i * B_LOC : (i + 1) * B_LOC]
        xb = np.concatenate(
            [sh[c * 128 : (c + 1) * 128] for c in range(C)], axis=1
        )
        blocks.append(np.ascontiguousarray(xb))
    return blocks, e_plus


def _get_nc(dt_x_name: str = "float16"):
    key = (dt_x_name, USE_ACT)
    if key not in _BUILD_CACHE:
        _BUILD_CACHE[key] = _build(dt_x_name, use_act=USE_ACT)
    return _BUILD_CACHE[key]


def _pack_inputs(x, p_w, np_dt):
    """Per-core [128, XCOLS] blocks: row p = [x[b=p] | x[b=128+p] | 0.5*p_w tiled]."""
    xs = x.reshape(B, FE).astype(np_dt)
    w = np.tile((0.5 * p_w.reshape(E)).astype(np_dt), C)      # [C*E]
    pw_block = np.broadcast_to(w, (128, C * E))
    blocks = []
    for i in range(N_CORES):
        sh = xs[i * B_LOC : (i + 1) * B_LOC]                  # [256, FE]
        xb = np.concatenate(
            [sh[c * 128 : (c + 1) * 128] for c in range(C)] + [pw_block], axis=1
        )
        blocks.append(np.ascontiguousarray(xb))
    return blocks


VARIANT = "fold"


def kernel(x, W_w, W_b, h_w, h_b, p_w, _trace: bool = False, _dt: str = "float16"):
    from concourse.bass_utils import run_bass_kernel_spmd

    x = np.asarray(x, dtype=np.float32)
    p_w = np.asarray(p_w, dtype=np.float32)

    np_dt = {"float16": np.float16, "float32": np.float32}[_dt]
    if VARIANT == "fold":
        blocks, e_plus = _pack_inputs_fold(x, p_w, np_dt)
        key = ("fold", _dt, e_plus)
        if key not in _BUILD_CACHE:
            _BUILD_CACHE[key] = _build_fold(e_plus, _dt, act_cols=32)
        nc = _BUILD_CACHE[key]
    else:
        blocks = _pack_inputs(x, p_w, np_dt)
        nc = _get_nc(_dt)
    in_maps = [{"xin": blocks[i]} for i in range(N_CORES)]
    r = run_bass_kernel_spmd(nc, in_maps, list(range(N_CORES)), trace=_trace)

    out = np.empty((B,), dtype=np.float32)
    for i in range(N_CORES):
        res = np.asarray(r.results[i]["out"])                 # [128, C]
        out[i * B_LOC : (i + 1) * B_LOC] = res.T.reshape(-1)
    if _trace:
        kernel.last_result = r
    return out.reshape(B, 1)


# revision 38
# speedup vs baseline: 1.0702x; 1.0702x over previous
"""AFM (attentional factorization machine) kernel for 8 TRN2 NeuronCores.

Math: the reference applies softmax over a size-1 axis => att == 1.0
exactly, so the attention MLP (W_w, W_b, h_w, h_b) is dead code and

    out[b] = sum_{i<j} (x[b,i,:] * x[b,j,:]) @ p_w
           = 0.5 * ((sum_f x[b,f,:])^2 - sum_f x[b,f,:]^2) @ p_w

Sharding: pure data parallel on batch (2048 -> 8 x 256); the tiny p_w is
folded into the input stream. Shipped variant ("fold", `_build_fold`):

  Host prep per core: x' = x * sqrt(0.5*|p_w[e]|) (fp16, halves HBM
  traffic; rel err ~1e-3 vs the 2e-2 gate), e-columns permuted so the
  p_w>=0 group comes first. Then with S' = sum_f x':

      out[b] = [sum_{e+} S'^2 - sum_{e-} S'^2]
             - [sum_{f,e+} x'^2 - sum_{f,e-} x'^2]

  Device (layout [128 partitions = batch, free = (c=2 groups, f=32, e=64)]):
    - 2 fp16 DMAs (one per batch group, compute starts after the first)
    - ScalarE: Square+accum_out over the first 32 e-cols of each sign
      group (the whole Q reduction fused into 4 ACT ops)
    - VectorE: 5-level halving tree over f (2x_1p fp16), S'^2, sign-group
      reduces, remainder Q columns (mult+reduce), final combine
    - out: [128, 2] f32 per core, host reassembles to [2048, 1]

  ~4.2-5.4 us steady-state per body on 8 cores (DMA floor 2.44 us).

This walrus build caps sync waits at 1/instruction (2 for EventSemaphore);
Tile emits more. Two TileContext patches fix that: excess waits hoisted
onto same-engine NoOps before lowering, and the kernel-tail drain split
into one single-wait pre-drain per proc.
"""

import numpy as np

N_CORES = 8
B = 2048
F = 32
E = 64
B_LOC = B // N_CORES        # 256
C = B_LOC // 128            # 2 batch groups per core
FE = F * E                  # 2048
XCOLS = C * FE + C * E      # 4352: x block + pw block

_BUILD_CACHE: dict = {}


def _build(
    dt_x_name: str = "float16",
    use_act: bool = True,
    repeat: int = 1,
    loopable: bool | str = False,
    bufs: int | None = None,
    probe: str | None = None,
    gq1: bool = False,
):
    import concourse.bass as bass
    import concourse.mybir as mybir
    import concourse.tile as tile

    dt_x = getattr(mybir.dt, dt_x_name)
    f32 = mybir.dt.float32

    import types

    from concourse.tile_scheduler import N_PROCS
    from concourse.vector_clock import ScopedClock, VectorClock

    nc = bass.Bass()
    x_p = nc.declare_dram_parameter("xin", [128, XCOLS], dt_x, isOutput=False)
    out_p = nc.declare_dram_parameter("out", [128, C], f32, isOutput=True)

    def _split_drain_and_barrier(self, tick_clock, wait_clock):
        # Walrus rejects >~2 sync-wait commands on one instruction; stock
        # Tile attaches every outstanding proc's sem to the single final
        # drain. Pre-drain once per proc (1 wait each); add_sem_waits'
        # per-engine elision then leaves the real drain with no new waits.
        snc = self.nc
        gc = tick_clock.global_clock
        for p in range(N_PROCS):
            if gc[p] > 0:
                vc = VectorClock()
                vc.require_at_least(p, gc[p])
                d = snc.sync.drain()
                wait_clock.add_sem_waits(d.ins, ScopedClock({None: vc}))
        # The pre-drains above already blocked SP on every outstanding sem;
        # SP program order makes further waits on the final drain redundant.
        snc.sync.drain()
        snc.all_engine_barrier()
        assert self.sems is not None
        popped = snc._tile_sem_poison_stack.pop()
        assert popped is self._sem_poison
        snc.clear_and_free_semaphores(list(self.sems.allocated().values()))
        snc.all_engine_barrier()

    def _make_split_excess_waits(tc):
        # Walrus enforces <=1 sync wait per instruction (2 for
        # EventSemaphore); Tile's wait assignment does not. Before commit,
        # hoist excess waits onto same-engine NoOps inserted just ahead.
        orig_lower = tc._lower_ordered_insts

        def wrapper(ordered):
            fixed = {}
            for bb_name, insts in ordered.items():
                new_list = []
                for inst in insts:
                    si = inst.sync_info
                    cap = 2 if isinstance(inst, mybir.InstEventSemaphore) else 1
                    if si is not None and si.on_wait and len(si.on_wait) > cap:
                        waits = list(si.on_wait)
                        keep, excess = waits[-cap:], waits[:-cap]
                        for w in excess:
                            nop = mybir.InstNoOp(
                                name=f"I-{nc.next_id()}", ins=[], outs=[]
                            )
                            nop.engine = inst.engine
                            nop.sync_info = mybir.SyncInfo(
                                on_wait=[w], on_update=[]
                            )
                            for attr in (
                                "bass_scheduled_proc",
                                "bass_scheduled_tick",
                                "bass_scheduled_scope",
                                "bass_priority",
                            ):
                                try:
                                    setattr(nop, attr, getattr(inst, attr))
                                except Exception:
                                    pass
                            new_list.append(nop)
                        inst.sync_info = mybir.SyncInfo(
                            on_wait=keep, on_update=list(si.on_update or [])
                        )
                    new_list.append(inst)
                fixed[bb_name] = new_list
            return orig_lower(fixed)

        return wrapper

    tc_ctx = tile.TileContext(nc)
    tc_ctx._drain_and_barrier = types.MethodType(_split_drain_and_barrier, tc_ctx)
    tc_ctx._lower_ordered_insts = _make_split_excess_waits(tc_ctx)

    if bufs is None:
        bufs = 3 if repeat > 1 else 1
    R_HALF = 0.7071067811865476  # ACT free-affine scale: Square(r*u) = u^2/2

    with tc_ctx as tc:
        with tc.tile_pool(name="main", bufs=bufs) as pool:
            for it in range(repeat):
                X = pool.tile([128, XCOLS], dt_x, tag="X")
                nc.sync.dma_start(out=X[:], in_=x_p[:])
                PW = X[:, C * FE : XCOLS]                  # [128, C*E] fp16

                def view(ap, f):
                    return ap.rearrange("p (c f e) -> p c f e", c=C, f=f, e=E)

                def tree_levels(src_ap, f0, tag, first_eng=None):
                    """Halving adds over f (f0 -> 1); optionally the first
                    level on another engine (gpsimd) to offload DVE."""
                    cur, f = src_ap, f0
                    while f > 1:
                        h = f // 2
                        nxt = pool.tile([128, C * h * E], dt_x, tag=f"{tag}{h}")
                        cv, nv = view(cur, f), view(nxt[:], h)
                        eng = first_eng if (f == f0 and first_eng) else nc.vector
                        eng.tensor_add(nv, cv[:, :, 0:h, :], cv[:, :, h:f, :])
                        cur, f = nxt[:], h
                    return cur

                Xv = view(X[:, 0 : C * FE], F)
                if probe == "dmaonly":
                    res = pool.tile([128, C], f32, tag="res")
                    nc.vector.tensor_reduce(
                        res[:],
                        X[:, 0 : C * E].rearrange("p (c e) -> p c e", c=C, e=E),
                        axis=mybir.AxisListType.X,
                        op=mybir.AluOpType.add,
                    )
                    if it == repeat - 1:
                        nc.sync.dma_start(out=out_p[:], in_=res[:])
                    continue
                if loopable == "cat":
                    # Concatenated trees: s1 and q1 written into one tile so
                    # levels 2..5 process both with single ops.
                    if probe == "nosq":
                        SQ = X  # timing probe: skip the ACT square
                    else:
                        SQ = pool.tile([128, C * FE], dt_x, tag="SQ")
                        nc.scalar.activation(
                            SQ[:], X[:, 0 : C * FE],
                            mybir.ActivationFunctionType.Square,
                        )
                    SQv = view(SQ[:, 0 : C * FE], F)
                    H = F // 2
                    seg = C * H * E                       # 2048
                    L = pool.tile([128, 2 * seg], dt_x, tag="L1")
                    nc.vector.tensor_add(
                        view(L[:, 0:seg], H), Xv[:, :, 0:H, :], Xv[:, :, H:F, :]
                    )
                    nc.vector.tensor_add(
                        view(L[:, seg : 2 * seg], H),
                        SQv[:, :, 0:H, :], SQv[:, :, H:F, :],
                    )
                    cur, f = L, H
                    while f > 1:
                        h = f // 2
                        nxt = pool.tile([128, 2 * C * h * E], dt_x, tag=f"L{h}")
                        cv = cur[:].rearrange(
                            "p (g c f e) -> p g c f e", g=2, c=C, f=f, e=E
                        )
                        nv = nxt[:].rearrange(
                            "p (g c f e) -> p g c f e", g=2, c=C, f=h, e=E
                        )
                        nc.vector.tensor_add(
                            nv, cv[:, :, :, 0:h, :], cv[:, :, :, h:f, :]
                        )
                        cur, f = nxt, h
                    S = cur[:, 0 : C * E]
                    Q = cur[:, C * E : 2 * C * E]
                elif loopable:
                    # u/v form: X is read only by DVE; ACT reads DVE-made
                    # tiles. u=x_lo+x_hi, v=x_lo-x_hi; (u^2+v^2)/2 = q1.
                    H = F // 2
                    s1 = pool.tile([128, C * H * E], dt_x, tag="s1")
                    v1 = pool.tile([128, C * H * E], dt_x, tag="v1")
                    nc.vector.tensor_add(view(s1[:], H), Xv[:, :, 0:H, :], Xv[:, :, H:F, :])
                    nc.vector.tensor_sub(view(v1[:], H), Xv[:, :, 0:H, :], Xv[:, :, H:F, :])
                    qa = pool.tile([128, C * H * E], dt_x, tag="qa")
                    qb = pool.tile([128, C * H * E], dt_x, tag="qb")
                    nc.scalar.activation(
                        qa[:], s1[:], mybir.ActivationFunctionType.Square, scale=R_HALF
                    )
                    nc.scalar.activation(
                        qb[:], v1[:], mybir.ActivationFunctionType.Square, scale=R_HALF
                    )
                    q1 = pool.tile([128, C * H * E], dt_x, tag="q1")
                    nc.vector.tensor_add(q1[:], qa[:], qb[:])
                    S = tree_levels(s1[:], H, "s")
                    Q = tree_levels(q1[:], H, "q")
                else:
                    SQ = pool.tile([128, C * FE], dt_x, tag="SQ")
                    for c in range(C):
                        if use_act:
                            nc.scalar.activation(
                                SQ[:, c * FE : (c + 1) * FE],
                                X[:, c * FE : (c + 1) * FE],
                                mybir.ActivationFunctionType.Square,
                            )
                        else:
                            nc.vector.tensor_mul(
                                SQ[:, c * FE : (c + 1) * FE],
                                X[:, c * FE : (c + 1) * FE],
                                X[:, c * FE : (c + 1) * FE],
                            )
                    S = tree_levels(X[:, 0 : C * FE], F, "s")
                    Q = tree_levels(
                        SQ[:], F, "q", first_eng=nc.gpsimd if gq1 else None
                    )

                # fp16 tail: res[p, c] = sum_e (S^2 - Q) * (0.5 * p_w)
                T = pool.tile([128, C * E], dt_x, tag="T")
                V = pool.tile([128, C * E], dt_x, tag="V")
                W = pool.tile([128, C * E], dt_x, tag="W")
                res = pool.tile([128, C], f32, tag="res")
                nc.vector.tensor_mul(T[:], S, S)
                nc.vector.tensor_sub(V[:], T[:], Q)
                nc.vector.tensor_mul(W[:], V[:], PW)
                nc.vector.tensor_reduce(
                    res[:],
                    W[:].rearrange("p (c e) -> p c e", c=C, e=E),
                    axis=mybir.AxisListType.X,
                    op=mybir.AluOpType.add,
                )
                if it == repeat - 1:
                    nc.sync.dma_start(out=out_p[:], in_=res[:])

    return nc


USE_ACT = True


def _build_fold(
    e_plus: int,
    dt_x_name: str = "float16",
    repeat: int = 1,
    bufs: int | None = None,
    act_cols: int | None = None,
):
    """pw-folded variant.

    Host pre-scales x by w[e] = sqrt(0.5*|p_w[e]|) and permutes e-columns so
    the pw>=0 group (e_plus columns) comes first. Then with x' the scaled
    input, S' = sum_f x':

        out[b] = sum_{e in +} (S'^2 + ... ) :
        out[b] = [sum_+ S'^2 - sum_- S'^2] - [sum_+ x'^2 - sum_- x'^2]

    The second bracket (Q-dot) is computed entirely on ScalarE via
    Square+accum_out; the first on DVE (tree + 2 chained TTRs per group).
    `act_cols` (multiple of 16, <=64) limits how many e-columns ACT handles
    per c; the rest go to DVE as chained TTRs (load balance).
    """
    import types

    import concourse.bass as bass
    import concourse.mybir as mybir
    import concourse.tile as tile
    from concourse.tile_scheduler import N_PROCS
    from concourse.vector_clock import ScopedClock, VectorClock

    dt_x = getattr(mybir.dt, dt_x_name)
    f32 = mybir.dt.float32
    XC = C * FE

    nc = bass.Bass()
    x_p = nc.declare_dram_parameter("xin", [128, XC], dt_x, isOutput=False)
    out_p = nc.declare_dram_parameter("out", [128, C], f32, isOutput=True)

    def _split_drain_and_barrier(self, tick_clock, wait_clock):
        snc = self.nc
        gc = tick_clock.global_clock
        for p in range(N_PROCS):
            if gc[p] > 0:
                vc = VectorClock()
                vc.require_at_least(p, gc[p])
                d = snc.sync.drain()
                wait_clock.add_sem_waits(d.ins, ScopedClock({None: vc}))
        snc.sync.drain()
        snc.all_engine_barrier()
        assert self.sems is not None
        popped = snc._tile_sem_poison_stack.pop()
        assert popped is self._sem_poison
        snc.clear_and_free_semaphores(list(self.sems.allocated().values()))
        snc.all_engine_barrier()

    def _make_split_excess_waits(tc):
        orig_lower = tc._lower_ordered_insts

        def wrapper(ordered):
            fixed = {}
            for bb_name, insts in ordered.items():
                new_list = []
                for inst in insts:
                    si = inst.sync_info
                    cap = 2 if isinstance(inst, mybir.InstEventSemaphore) else 1
                    if si is not None and si.on_wait and len(si.on_wait) > cap:
                        waits = list(si.on_wait)
                        keep, excess = waits[-cap:], waits[:-cap]
                        for w in excess:
                            nop = mybir.InstNoOp(
                                name=f"I-{nc.next_id()}", ins=[], outs=[]
                            )
                            nop.engine = inst.engine
                            nop.sync_info = mybir.SyncInfo(on_wait=[w], on_update=[])
                            for attr in (
                                "bass_scheduled_proc",
                                "bass_scheduled_tick",
                                "bass_scheduled_scope",
                                "bass_priority",
                            ):
                                try:
                                    setattr(nop, attr, getattr(inst, attr))
                                except Exception:
                                    pass
                            new_list.append(nop)
                        inst.sync_info = mybir.SyncInfo(
                            on_wait=keep, on_update=list(si.on_update or [])
                        )
                    new_list.append(inst)
                fixed[bb_name] = new_list
            return orig_lower(fixed)

        return wrapper

    if bufs is None:
        bufs = 3 if repeat > 1 else 1
    if act_cols is None:
        act_cols = E

    tc_ctx = tile.TileContext(nc)
    tc_ctx._drain_and_barrier = types.MethodType(_split_drain_and_barrier, tc_ctx)
    tc_ctx._lower_ordered_insts = _make_split_excess_waits(tc_ctx)

    AL = mybir.AluOpType

    with tc_ctx as tc:
        with tc.tile_pool(name="main", bufs=bufs) as pool:
            for it in range(repeat):
                X = pool.tile([128, XC], dt_x, tag="X")
                # split by batch group so group-0 compute overlaps group-1 DMA
                for c in range(C):
                    nc.sync.dma_start(
                        out=X[:, c * FE : (c + 1) * FE],
                        in_=x_p[:, c * FE : (c + 1) * FE],
                    )

                JKA = pool.tile([128, FE], dt_x, tag="JKA")     # ACT junk out
                JKD = pool.tile([128, FE], dt_x, tag="JKD")     # DVE Q junk out

                # Contributions land as [128, C] strips in POS/NEG tiles:
                #   POS: +S'^2 pos-group, +Q neg-group
                #   NEG: +S'^2 neg-group, +Q pos-group
                # res = reduce(POS) - reduce(NEG).
                pos_strips: list = []
                neg_strips: list = []
                POS = pool.tile([128, 3 * C], f32, tag="POS")
                NEG = pool.tile([128, 3 * C], f32, tag="NEG")

                def strip(which):
                    lst, t = (
                        (pos_strips, POS) if which == "pos" else (neg_strips, NEG)
                    )
                    i = len(lst)
                    ap = t[:, i * C : (i + 1) * C]
                    lst.append(ap)
                    return ap

                # --- Q-dot: per (c, sign group): first act_cols columns on
                # ScalarE (Square+accum_out), remainder on DVE (mult+reduce).
                # One strip per (group, engine); column c written per c.
                e_neg = E - e_plus
                groups = []  # (g0, g1, which)
                groups.append((0, e_plus, "neg"))      # +Qpos contributes -
                if e_neg > 0:
                    groups.append((e_plus, E, "pos"))  # +Qneg contributes +
                gstrips = {}
                for gi, (g0, g1, which) in enumerate(groups):
                    na = min(act_cols, g1 - g0)
                    gstrips[gi] = (
                        strip(which) if na > 0 else None,
                        strip(which) if g0 + na < g1 else None,
                        na,
                    )
                for c in range(C):
                    xv = X[:, c * FE : (c + 1) * FE].rearrange(
                        "p (f e) -> p f e", f=F, e=E
                    )
                    for gi, (g0, g1, which) in enumerate(groups):
                        st_a, st_d, na = gstrips[gi]
                        if st_a is not None:
                            nc.scalar.activation(
                                JKA[:, 0 : F * na],
                                xv[:, :, g0 : g0 + na],
                                mybir.ActivationFunctionType.Square,
                                accum_out=st_a[:, c : c + 1],
                            )
                        if st_d is not None:
                            nd = g1 - (g0 + na)
                            sqr = JKD[:, 0 : F * nd]
                            nc.vector.tensor_mul(
                                sqr.rearrange("p (f e) -> p f e", f=F, e=nd),
                                xv[:, :, g0 + na : g1],
                                xv[:, :, g0 + na : g1],
                            )
                            nc.vector.tensor_reduce(
                                st_d[:, c : c + 1],
                                sqr.rearrange("p (f e) -> p f e", f=F, e=nd),
                                axis=mybir.AxisListType.XY,
                                op=AL.add,
                            )

                # --- S' tree on DVE: level 1 split per c (starts right
                # after that group's DMA), levels 2..5 c-joint.
                def view(ap, f):
                    return ap.rearrange("p (c f e) -> p c f e", c=C, f=f, e=E)

                H1 = F // 2
                s1 = pool.tile([128, C * H1 * E], dt_x, tag="s1")
                for c in range(C):
                    xc = X[:, c * FE : (c + 1) * FE].rearrange(
                        "p (f e) -> p f e", f=F, e=E
                    )
                    s1c = s1[:, c * H1 * E : (c + 1) * H1 * E].rearrange(
                        "p (f e) -> p f e", f=H1, e=E
                    )
                    nc.vector.tensor_add(s1c, xc[:, 0:H1, :], xc[:, H1:F, :])
                cur, f = s1[:], H1
                while f > 1:
                    h = f // 2
                    nxt = pool.tile([128, C * h * E], dt_x, tag=f"s{h}")
                    nc.vector.tensor_add(
                        view(nxt[:], h),
                        view(cur, f)[:, :, 0:h, :],
                        view(cur, f)[:, :, h:f, :],
                    )
                    cur, f = nxt[:], h
                S = cur  # [128, C*E] = sqrt(.5|pw|)*S, e-permuted

                # --- S-dot: T = S'*S' (f32), then per-sign-group reduces.
                T = pool.tile([128, C * E], f32, tag="T")
                nc.vector.tensor_mul(T[:], S, S)
                Tv = T[:].rearrange("p (c e) -> p c e", c=C, e=E)
                if e_plus > 0:
                    sp = strip("pos")
                    nc.vector.tensor_reduce(
                        sp, Tv[:, :, 0:e_plus], axis=mybir.AxisListType.X,
                        op=AL.add,
                    )
                if e_neg > 0:
                    sn = strip("neg")
                    nc.vector.tensor_reduce(
                        sn, Tv[:, :, e_plus:E], axis=mybir.AxisListType.X,
                        op=AL.add,
                    )

                # --- combine: res = rowsum(POS) - rowsum(NEG)
                res = pool.tile([128, C], f32, tag="res")
                RP = pool.tile([128, C], f32, tag="RP")
                RN = pool.tile([128, C], f32, tag="RN")
                np_, nn_ = len(pos_strips), len(neg_strips)

                def rowsum(dst, t, n):
                    if n == 0:
                        nc.vector.memset(dst, 0.0)
                        return
                    v = t[:, 0 : n * C].rearrange("p (r c) -> p c r", r=n, c=C)
                    nc.vector.tensor_reduce(
                        dst, v, axis=mybir.AxisListType.X, op=AL.add
                    )

                rowsum(RP[:], POS, np_)
                rowsum(RN[:], NEG, nn_)
                nc.vector.tensor_sub(res[:], RP[:], RN[:])
                if it == repeat - 1:
                    nc.sync.dma_start(out=out_p[:], in_=res[:])

    return nc


def _pack_inputs_fold(x, p_w, np_dt):
    """Per-core [128, C*FE] blocks of x scaled by sqrt(.5|pw|), e-sorted."""
    pw = p_w.reshape(E).astype(np.float64)
    perm = np.argsort(pw < 0, kind="stable")       # pw>=0 first
    e_plus = int((pw >= 0).sum())
    w = np.sqrt(0.5 * np.abs(pw[perm]))
    xs = x.reshape(B, F, E)[:, :, perm] * w[None, None, :]
    xs = xs.reshape(B, FE).astype(np_dt)
    blocks = []
    for i in range(N_CORES):
        sh = xs[i * B_LOC : (i + 1) * B_LOC]
        xb = np.concatenate(
            [sh[c * 128 : (c + 1) * 128] for c in range(C)], axis=1
        )
        blocks.append(np.ascontiguousarray(xb))
    return blocks, e_plus


def _get_nc(dt_x_name: str = "float16"):
    key = (dt_x_name, USE_ACT)
    if key not in _BUILD_CACHE:
        _BUILD_CACHE[key] = _build(dt_x_name, use_act=USE_ACT)
    return _BUILD_CACHE[key]


def _pack_inputs(x, p_w, np_dt):
    """Per-core [128, XCOLS] blocks: row p = [x[b=p] | x[b=128+p] | 0.5*p_w tiled]."""
    xs = x.reshape(B, FE).astype(np_dt)
    w = np.tile((0.5 * p_w.reshape(E)).astype(np_dt), C)      # [C*E]
    pw_block = np.broadcast_to(w, (128, C * E))
    blocks = []
    for i in range(N_CORES):
        sh = xs[i * B_LOC : (i + 1) * B_LOC]                  # [256, FE]
        xb = np.concatenate(
            [sh[c * 128 : (c + 1) * 128] for c in range(C)] + [pw_block], axis=1
        )
        blocks.append(np.ascontiguousarray(xb))
    return blocks


VARIANT = "fold"


def kernel(x, W_w, W_b, h_w, h_b, p_w, _trace: bool = False, _dt: str = "float16"):
    from concourse.bass_utils import run_bass_kernel_spmd

    x = np.asarray(x, dtype=np.float32)
    p_w = np.asarray(p_w, dtype=np.float32)

    np_dt = {"float16": np.float16, "float32": np.float32}[_dt]
    if VARIANT == "fold":
        blocks, e_plus = _pack_inputs_fold(x, p_w, np_dt)
        key = ("fold", _dt, e_plus)
        if key not in _BUILD_CACHE:
            _BUILD_CACHE[key] = _build_fold(e_plus, _dt, act_cols=32)
        nc = _BUILD_CACHE[key]
    else:
        blocks = _pack_inputs(x, p_w, np_dt)
        nc = _get_nc(_dt)
    in_maps = [{"xin": blocks[i]} for i in range(N_CORES)]
    r = run_bass_kernel_spmd(nc, in_maps, list(range(N_CORES)), trace=_trace)

    out = np.empty((B,), dtype=np.float32)
    for i in range(N_CORES):
        res = np.asarray(r.results[i]["out"])                 # [128, C]
        out[i * B_LOC : (i + 1) * B_LOC] = res.T.reshape(-1)
    if _trace:
        kernel.last_result = r
    return out.reshape(B, 1)


# revision 53
# speedup vs baseline: 1.1211x; 1.0476x over previous
"""AFM (attentional factorization machine) kernel for 8 TRN2 NeuronCores.

Math: the reference applies softmax over a size-1 axis => att == 1.0
exactly, so the attention MLP (W_w, W_b, h_w, h_b) is dead code and

    out[b] = sum_{i<j} (x[b,i,:] * x[b,j,:]) @ p_w
           = 0.5 * ((sum_f x[b,f,:])^2 - sum_f x[b,f,:]^2) @ p_w

Sharding: pure data parallel on batch (2048 -> 8 x 256); the tiny p_w is
folded into the input stream. Shipped variant ("fold", `_build_fold`):

  Host prep per core: x' = x * sqrt(0.5*|p_w[e]|) (fp16, halves HBM
  traffic; rel err ~1e-3 vs the 2e-2 gate), e-columns permuted so the
  p_w>=0 group comes first. Then with S' = sum_f x':

      out[b] = [sum_{e+} S'^2 - sum_{e-} S'^2]
             - [sum_{f,e+} x'^2 - sum_{f,e-} x'^2]

  Device (layout [128 partitions = batch, free = (c=2 groups, f=32, e=64)]):
    - 2 fp16 DMAs (one per batch group, compute starts after the first)
    - ScalarE: Square+accum_out over the first 32 e-cols of each sign
      group (the whole Q reduction fused into 4 ACT ops)
    - VectorE: 5-level halving tree over f (2x_1p fp16), S'^2, sign-group
      reduces, remainder Q columns (mult+reduce), final combine
    - out: [128, 2] f32 per core, host reassembles to [2048, 1]

  ~4.2-5.4 us steady-state per body on 8 cores (DMA floor 2.44 us).

This walrus build caps sync waits at 1/instruction (2 for EventSemaphore);
Tile emits more. Two TileContext patches fix that: excess waits hoisted
onto same-engine NoOps before lowering, and the kernel-tail drain split
into one single-wait pre-drain per proc.
"""

import numpy as np

N_CORES = 8
B = 2048
F = 32
E = 64
B_LOC = B // N_CORES        # 256
C = B_LOC // 128            # 2 batch groups per core
FE = F * E                  # 2048
XCOLS = C * FE + C * E      # 4352: x block + pw block

_BUILD_CACHE: dict = {}


def _build(
    dt_x_name: str = "float16",
    use_act: bool = True,
    repeat: int = 1,
    loopable: bool | str = False,
    bufs: int | None = None,
    probe: str | None = None,
    gq1: bool = False,
):
    import concourse.bass as bass
    import concourse.mybir as mybir
    import concourse.tile as tile

    dt_x = getattr(mybir.dt, dt_x_name)
    f32 = mybir.dt.float32

    import types

    from concourse.tile_scheduler import N_PROCS
    from concourse.vector_clock import ScopedClock, VectorClock

    nc = bass.Bass()
    x_p = nc.declare_dram_parameter("xin", [128, XCOLS], dt_x, isOutput=False)
    out_p = nc.declare_dram_parameter("out", [128, C], f32, isOutput=True)

    def _split_drain_and_barrier(self, tick_clock, wait_clock):
        # Walrus rejects >~2 sync-wait commands on one instruction; stock
        # Tile attaches every outstanding proc's sem to the single final
        # drain. Pre-drain once per proc (1 wait each); add_sem_waits'
        # per-engine elision then leaves the real drain with no new waits.
        snc = self.nc
        gc = tick_clock.global_clock
        for p in range(N_PROCS):
            if gc[p] > 0:
                vc = VectorClock()
                vc.require_at_least(p, gc[p])
                d = snc.sync.drain()
                wait_clock.add_sem_waits(d.ins, ScopedClock({None: vc}))
        # The pre-drains above already blocked SP on every outstanding sem;
        # SP program order makes further waits on the final drain redundant.
        snc.sync.drain()
        snc.all_engine_barrier()
        assert self.sems is not None
        popped = snc._tile_sem_poison_stack.pop()
        assert popped is self._sem_poison
        snc.clear_and_free_semaphores(list(self.sems.allocated().values()))
        snc.all_engine_barrier()

    def _make_split_excess_waits(tc):
        # Walrus enforces <=1 sync wait per instruction (2 for
        # EventSemaphore); Tile's wait assignment does not. Before commit,
        # hoist excess waits onto same-engine NoOps inserted just ahead.
        orig_lower = tc._lower_ordered_insts

        def wrapper(ordered):
            fixed = {}
            for bb_name, insts in ordered.items():
                new_list = []
                for inst in insts:
                    si = inst.sync_info
                    cap = 2 if isinstance(inst, mybir.InstEventSemaphore) else 1
                    if si is not None and si.on_wait and len(si.on_wait) > cap:
                        waits = list(si.on_wait)
                        keep, excess = waits[-cap:], waits[:-cap]
                        for w in excess:
                            nop = mybir.InstNoOp(
                                name=f"I-{nc.next_id()}", ins=[], outs=[]
                            )
                            nop.engine = inst.engine
                            nop.sync_info = mybir.SyncInfo(
                                on_wait=[w], on_update=[]
                            )
                            for attr in (
                                "bass_scheduled_proc",
                                "bass_scheduled_tick",
                                "bass_scheduled_scope",
                                "bass_priority",
                            ):
                                try:
                                    setattr(nop, attr, getattr(inst, attr))
                                except Exception:
                                    pass
                            new_list.append(nop)
                        inst.sync_info = mybir.SyncInfo(
                            on_wait=keep, on_update=list(si.on_update or [])
                        )
                    new_list.append(inst)
                fixed[bb_name] = new_list
            return orig_lower(fixed)

        return wrapper

    tc_ctx = tile.TileContext(nc)
    tc_ctx._drain_and_barrier = types.MethodType(_split_drain_and_barrier, tc_ctx)
    tc_ctx._lower_ordered_insts = _make_split_excess_waits(tc_ctx)

    if bufs is None:
        bufs = 3 if repeat > 1 else 1
    R_HALF = 0.7071067811865476  # ACT free-affine scale: Square(r*u) = u^2/2

    with tc_ctx as tc:
        with tc.tile_pool(name="main", bufs=bufs) as pool:
            for it in range(repeat):
                X = pool.tile([128, XCOLS], dt_x, tag="X")
                nc.sync.dma_start(out=X[:], in_=x_p[:])
                PW = X[:, C * FE : XCOLS]                  # [128, C*E] fp16

                def view(ap, f):
                    return ap.rearrange("p (c f e) -> p c f e", c=C, f=f, e=E)

                def tree_levels(src_ap, f0, tag, first_eng=None):
                    """Halving adds over f (f0 -> 1); optionally the first
                    level on another engine (gpsimd) to offload DVE."""
                    cur, f = src_ap, f0
                    while f > 1:
                        h = f // 2
                        nxt = pool.tile([128, C * h * E], dt_x, tag=f"{tag}{h}")
                        cv, nv = view(cur, f), view(nxt[:], h)
                        eng = first_eng if (f == f0 and first_eng) else nc.vector
                        eng.tensor_add(nv, cv[:, :, 0:h, :], cv[:, :, h:f, :])
                        cur, f = nxt[:], h
                    return cur

                Xv = view(X[:, 0 : C * FE], F)
                if probe == "dmaonly":
                    res = pool.tile([128, C], f32, tag="res")
                    nc.vector.tensor_reduce(
                        res[:],
                        X[:, 0 : C * E].rearrange("p (c e) -> p c e", c=C, e=E),
                        axis=mybir.AxisListType.X,
                        op=mybir.AluOpType.add,
                    )
                    if it == repeat - 1:
                        nc.sync.dma_start(out=out_p[:], in_=res[:])
                    continue
                if loopable == "cat":
                    # Concatenated trees: s1 and q1 written into one tile so
                    # levels 2..5 process both with single ops.
                    if probe == "nosq":
                        SQ = X  # timing probe: skip the ACT square
                    else:
                        SQ = pool.tile([128, C * FE], dt_x, tag="SQ")
                        nc.scalar.activation(
                            SQ[:], X[:, 0 : C * FE],
                            mybir.ActivationFunctionType.Square,
                        )
                    SQv = view(SQ[:, 0 : C * FE], F)
                    H = F // 2
                    seg = C * H * E                       # 2048
                    L = pool.tile([128, 2 * seg], dt_x, tag="L1")
                    nc.vector.tensor_add(
                        view(L[:, 0:seg], H), Xv[:, :, 0:H, :], Xv[:, :, H:F, :]
                    )
                    nc.vector.tensor_add(
                        view(L[:, seg : 2 * seg], H),
                        SQv[:, :, 0:H, :], SQv[:, :, H:F, :],
                    )
                    cur, f = L, H
                    while f > 1:
                        h = f // 2
                        nxt = pool.tile([128, 2 * C * h * E], dt_x, tag=f"L{h}")
                        cv = cur[:].rearrange(
                            "p (g c f e) -> p g c f e", g=2, c=C, f=f, e=E
                        )
                        nv = nxt[:].rearrange(
                            "p (g c f e) -> p g c f e", g=2, c=C, f=h, e=E
                        )
                        nc.vector.tensor_add(
                            nv, cv[:, :, :, 0:h, :], cv[:, :, :, h:f, :]
                        )
                        cur, f = nxt, h
                    S = cur[:, 0 : C * E]
                    Q = cur[:, C * E : 2 * C * E]
                elif loopable:
                    # u/v form: X is read only by DVE; ACT reads DVE-made
                    # tiles. u=x_lo+x_hi, v=x_lo-x_hi; (u^2+v^2)/2 = q1.
                    H = F // 2
                    s1 = pool.tile([128, C * H * E], dt_x, tag="s1")
                    v1 = pool.tile([128, C * H * E], dt_x, tag="v1")
                    nc.vector.tensor_add(view(s1[:], H), Xv[:, :, 0:H, :], Xv[:, :, H:F, :])
                    nc.vector.tensor_sub(view(v1[:], H), Xv[:, :, 0:H, :], Xv[:, :, H:F, :])
                    qa = pool.tile([128, C * H * E], dt_x, tag="qa")
                    qb = pool.tile([128, C * H * E], dt_x, tag="qb")
                    nc.scalar.activation(
                        qa[:], s1[:], mybir.ActivationFunctionType.Square, scale=R_HALF
                    )
                    nc.scalar.activation(
                        qb[:], v1[:], mybir.ActivationFunctionType.Square, scale=R_HALF
                    )
                    q1 = pool.tile([128, C * H * E], dt_x, tag="q1")
                    nc.vector.tensor_add(q1[:], qa[:], qb[:])
                    S = tree_levels(s1[:], H, "s")
                    Q = tree_levels(q1[:], H, "q")
                else:
                    SQ = pool.tile([128, C * FE], dt_x, tag="SQ")
                    for c in range(C):
                        if use_act:
                            nc.scalar.activation(
                                SQ[:, c * FE : (c + 1) * FE],
                                X[:, c * FE : (c + 1) * FE],
                                mybir.ActivationFunctionType.Square,
                            )
                        else:
                            nc.vector.tensor_mul(
                                SQ[:, c * FE : (c + 1) * FE],
                                X[:, c * FE : (c + 1) * FE],
                                X[:, c * FE : (c + 1) * FE],
                            )
                    S = tree_levels(X[:, 0 : C * FE], F, "s")
                    Q = tree_levels(
                        SQ[:], F, "q", first_eng=nc.gpsimd if gq1 else None
                    )

                # fp16 tail: res[p, c] = sum_e (S^2 - Q) * (0.5 * p_w)
                T = pool.tile([128, C * E], dt_x, tag="T")
                V = pool.tile([128, C * E], dt_x, tag="V")
                W = pool.tile([128, C * E], dt_x, tag="W")
                res = pool.tile([128, C], f32, tag="res")
                nc.vector.tensor_mul(T[:], S, S)
                nc.vector.tensor_sub(V[:], T[:], Q)
                nc.vector.tensor_mul(W[:], V[:], PW)
                nc.vector.tensor_reduce(
                    res[:],
                    W[:].rearrange("p (c e) -> p c e", c=C, e=E),
                    axis=mybir.AxisListType.X,
                    op=mybir.AluOpType.add,
                )
                if it == repeat - 1:
                    nc.sync.dma_start(out=out_p[:], in_=res[:])

    return nc


USE_ACT = True


def _build_fold(
    e_plus: int,
    dt_x_name: str = "float16",
    repeat: int = 1,
    bufs: int | None = None,
    act_cols: int | None = None,
    probe: str | None = None,   # "noact" | "nodve" (timing-only builds)
    jka_psum: bool = False,     # ACT junk output to PSUM (faster ScE port)
    f_inner: bool = False,      # x packed [c, e, f]: contiguous ACT reads
):
    """pw-folded variant.

    Host pre-scales x by w[e] = sqrt(0.5*|p_w[e]|) and permutes e-columns so
    the pw>=0 group (e_plus columns) comes first. Then with x' the scaled
    input, S' = sum_f x':

        out[b] = sum_{e in +} (S'^2 + ... ) :
        out[b] = [sum_+ S'^2 - sum_- S'^2] - [sum_+ x'^2 - sum_- x'^2]

    The second bracket (Q-dot) is computed entirely on ScalarE via
    Square+accum_out; the first on DVE (tree + 2 chained TTRs per group).
    `act_cols` (multiple of 16, <=64) limits how many e-columns ACT handles
    per c; the rest go to DVE as chained TTRs (load balance).
    """
    import types

    import concourse.bass as bass
    import concourse.mybir as mybir
    import concourse.tile as tile
    from concourse.tile_scheduler import N_PROCS
    from concourse.vector_clock import ScopedClock, VectorClock

    dt_x = getattr(mybir.dt, dt_x_name)
    f32 = mybir.dt.float32
    XC = C * FE

    nc = bass.Bass()
    x_p = nc.declare_dram_parameter("xin", [128, XC], dt_x, isOutput=False)
    out_p = nc.declare_dram_parameter("out", [128, C], f32, isOutput=True)

    def _split_drain_and_barrier(self, tick_clock, wait_clock):
        snc = self.nc
        gc = tick_clock.global_clock
        for p in range(N_PROCS):
            if gc[p] > 0:
                vc = VectorClock()
                vc.require_at_least(p, gc[p])
                d = snc.sync.drain()
                wait_clock.add_sem_waits(d.ins, ScopedClock({None: vc}))
        snc.sync.drain()
        snc.all_engine_barrier()
        assert self.sems is not None
        popped = snc._tile_sem_poison_stack.pop()
        assert popped is self._sem_poison
        snc.clear_and_free_semaphores(list(self.sems.allocated().values()))
        snc.all_engine_barrier()

    def _make_split_excess_waits(tc):
        orig_lower = tc._lower_ordered_insts

        def wrapper(ordered):
            fixed = {}
            for bb_name, insts in ordered.items():
                new_list = []
                for inst in insts:
                    si = inst.sync_info
                    cap = 2 if isinstance(inst, mybir.InstEventSemaphore) else 1
                    if si is not None and si.on_wait and len(si.on_wait) > cap:
                        waits = list(si.on_wait)
                        keep, excess = waits[-cap:], waits[:-cap]
                        for w in excess:
                            nop = mybir.InstNoOp(
                                name=f"I-{nc.next_id()}", ins=[], outs=[]
                            )
                            nop.engine = inst.engine
                            nop.sync_info = mybir.SyncInfo(on_wait=[w], on_update=[])
                            for attr in (
                                "bass_scheduled_proc",
                                "bass_scheduled_tick",
                                "bass_scheduled_scope",
                                "bass_priority",
                            ):
                                try:
                                    setattr(nop, attr, getattr(inst, attr))
                                except Exception:
                                    pass
                            new_list.append(nop)
                        inst.sync_info = mybir.SyncInfo(
                            on_wait=keep, on_update=list(si.on_update or [])
                        )
                    new_list.append(inst)
                fixed[bb_name] = new_list
            return orig_lower(fixed)

        return wrapper

    if bufs is None:
        bufs = 3 if repeat > 1 else 1
    if act_cols is None:
        act_cols = E

    tc_ctx = tile.TileContext(nc)
    tc_ctx._drain_and_barrier = types.MethodType(_split_drain_and_barrier, tc_ctx)
    tc_ctx._lower_ordered_insts = _make_split_excess_waits(tc_ctx)

    AL = mybir.AluOpType

    from contextlib import ExitStack

    with tc_ctx as tc, ExitStack() as _st:
        psum_pool = (
            _st.enter_context(tc.tile_pool(name="pjk", bufs=1, space="PSUM"))
            if jka_psum
            else None
        )
        with tc.tile_pool(name="main", bufs=bufs) as pool:
            for it in range(repeat):
                X = pool.tile([128, XC], dt_x, tag="X")
                # split by batch group so group-0 compute overlaps group-1 DMA
                for c in range(C):
                    nc.sync.dma_start(
                        out=X[:, c * FE : (c + 1) * FE],
                        in_=x_p[:, c * FE : (c + 1) * FE],
                    )

                if jka_psum:
                    JKA = psum_pool.tile([128, FE], f32, tag="JKA")
                else:
                    JKA = pool.tile([128, FE], dt_x, tag="JKA")  # ACT junk out
                JKD = pool.tile([128, FE], dt_x, tag="JKD")     # DVE Q junk out

                # Contributions land as [128, C] strips in POS/NEG tiles:
                #   POS: +S'^2 pos-group, +Q neg-group
                #   NEG: +S'^2 neg-group, +Q pos-group
                # res = reduce(POS) - reduce(NEG).
                pos_strips: list = []
                neg_strips: list = []
                POS = pool.tile([128, 3 * C], f32, tag="POS")
                NEG = pool.tile([128, 3 * C], f32, tag="NEG")

                def strip(which):
                    lst, t = (
                        (pos_strips, POS) if which == "pos" else (neg_strips, NEG)
                    )
                    i = len(lst)
                    ap = t[:, i * C : (i + 1) * C]
                    lst.append(ap)
                    return ap

                # --- Q-dot: per (c, sign group): first act_cols columns on
                # ScalarE (Square+accum_out), remainder on DVE (mult+reduce).
                # One strip per (group, engine); column c written per c.
                e_neg = E - e_plus
                groups = []  # (g0, g1, which)
                groups.append((0, e_plus, "neg"))      # +Qpos contributes -
                if e_neg > 0:
                    groups.append((e_plus, E, "pos"))  # +Qneg contributes +
                gstrips = {}
                if probe not in ("actcontig4", "actcontig2"):
                    for gi, (g0, g1, which) in enumerate(groups):
                        na = min(act_cols, g1 - g0)
                        gstrips[gi] = (
                            strip(which) if na > 0 else None,
                            strip(which) if g0 + na < g1 else None,
                            na,
                        )
                if probe in ("actcontig4", "actcontig2"):
                    # Timing probes: same ACT element counts/op counts but
                    # fully contiguous reads (math is wrong; timing valid).
                    for c in range(C):
                        base = c * FE
                        if probe == "actcontig4":
                            sizes = [F * e_plus, F * (E - e_plus)]
                        else:
                            sizes = [FE]
                        off = 0
                        for si, n in enumerate(sizes):
                            if n == 0:
                                continue
                            st = strip("neg" if si == 0 else "pos")
                            nc.scalar.activation(
                                JKA[:, 0:n],
                                X[:, base + off : base + off + n],
                                mybir.ActivationFunctionType.Square,
                                accum_out=st[:, c : c + 1],
                            )
                            off += n
                for c in range(C if probe not in ("actcontig4", "actcontig2") else 0):
                    base = c * FE
                    xv = X[:, base : base + FE].rearrange(
                        "p (f e) -> p f e", f=F, e=E
                    )
                    for gi, (g0, g1, which) in enumerate(groups):
                        st_a, st_d, na = gstrips[gi]
                        if st_a is not None and probe != "noact":
                            if f_inner:
                                # [c, e, f] layout: group is contiguous
                                src = X[:, base + g0 * F : base + (g0 + na) * F]
                            else:
                                src = xv[:, :, g0 : g0 + na]
                            nc.scalar.activation(
                                JKA[:, 0 : F * na],
                                src,
                                mybir.ActivationFunctionType.Square,
                                accum_out=st_a[:, c : c + 1],
                            )
                        if st_d is not None and probe != "nodve":
                            nd = g1 - (g0 + na)
                            sqr = JKD[:, 0 : F * nd]
                            if f_inner:
                                src = X[:, base + (g0 + na) * F : base + g1 * F]
                                nc.vector.tensor_mul(sqr, src, src)
                                nc.vector.tensor_reduce(
                                    st_d[:, c : c + 1],
                                    sqr,
                                    axis=mybir.AxisListType.X,
                                    op=AL.add,
                                )
                            else:
                                src = xv[:, :, g0 + na : g1]
                                nc.vector.tensor_mul(
                                    sqr.rearrange("p (f e) -> p f e", f=F, e=nd),
                                    src, src,
                                )
                                nc.vector.tensor_reduce(
                                    st_d[:, c : c + 1],
                                    sqr.rearrange("p (f e) -> p f e", f=F, e=nd),
                                    axis=mybir.AxisListType.XY,
                                    op=AL.add,
                                )

                # --- S' tree on DVE: level 1 split per c (starts right
                # after that group's DMA), levels 2..5 c-joint.
                def viewt(ap, f):
                    if f_inner:
                        return ap.rearrange("p (c e f) -> p c e f", c=C, e=E, f=f)
                    return ap.rearrange("p (c f e) -> p c f e", c=C, f=f, e=E)

                def halves(v, h, f):
                    if f_inner:
                        return v[:, :, :, 0:h], v[:, :, :, h:f]
                    return v[:, :, 0:h, :], v[:, :, h:f, :]

                if probe in ("nodve", "actcontig4", "actcontig2"):
                    # ACT+DMA-bound probe: skip the S path entirely.
                    res = pool.tile([128, C], f32, tag="res")
                    RP = pool.tile([128, C], f32, tag="RP")
                    RN = pool.tile([128, C], f32, tag="RN")
                    for dst, t, lst in ((RP, POS, pos_strips), (RN, NEG, neg_strips)):
                        n = len(lst)
                        if n == 0:
                            nc.vector.memset(dst[:], 0.0)
                            continue
                        v = t[:, 0 : n * C].rearrange(
                            "p (r c) -> p c r", r=n, c=C
                        )
                        nc.vector.tensor_reduce(
                            dst[:], v, axis=mybir.AxisListType.X, op=AL.add
                        )
                    nc.vector.tensor_sub(res[:], RP[:], RN[:])
                    if it == repeat - 1:
                        nc.sync.dma_start(out=out_p[:], in_=res[:])
                    continue

                H1 = F // 2
                s1 = pool.tile([128, C * H1 * E], dt_x, tag="s1")
                for c in range(C):
                    if f_inner:
                        xc = X[:, c * FE : (c + 1) * FE].rearrange(
                            "p (e f) -> p e f", e=E, f=F
                        )
                        s1c = s1[:, c * H1 * E : (c + 1) * H1 * E].rearrange(
                            "p (e f) -> p e f", e=E, f=H1
                        )
                        nc.vector.tensor_add(s1c, xc[:, :, 0:H1], xc[:, :, H1:F])
                    else:
                        xc = X[:, c * FE : (c + 1) * FE].rearrange(
                            "p (f e) -> p f e", f=F, e=E
                        )
                        s1c = s1[:, c * H1 * E : (c + 1) * H1 * E].rearrange(
                            "p (f e) -> p f e", f=H1, e=E
                        )
                        nc.vector.tensor_add(s1c, xc[:, 0:H1, :], xc[:, H1:F, :])
                cur, f = s1[:], H1
                while f > 1:
                    h = f // 2
                    nxt = pool.tile([128, C * h * E], dt_x, tag=f"s{h}")
                    cv, nv = viewt(cur, f), viewt(nxt[:], h)
                    a, b = halves(cv, h, f)
                    nc.vector.tensor_add(nv, a, b)
                    cur, f = nxt[:], h
                S = cur  # [128, C*E] = sqrt(.5|pw|)*S, e-permuted, e-inner

                # --- S-dot: T = S'*S' (f32), then per-sign-group reduces.
                T = pool.tile([128, C * E], f32, tag="T")
                nc.vector.tensor_mul(T[:], S, S)
                Tv = T[:].rearrange("p (c e) -> p c e", c=C, e=E)
                if e_plus > 0:
                    sp = strip("pos")
                    nc.vector.tensor_reduce(
                        sp, Tv[:, :, 0:e_plus], axis=mybir.AxisListType.X,
                        op=AL.add,
                    )
                if e_neg > 0:
                    sn = strip("neg")
                    nc.vector.tensor_reduce(
                        sn, Tv[:, :, e_plus:E], axis=mybir.AxisListType.X,
                        op=AL.add,
                    )

                # --- combine: res = rowsum(POS) - rowsum(NEG)
                res = pool.tile([128, C], f32, tag="res")
                RP = pool.tile([128, C], f32, tag="RP")
                RN = pool.tile([128, C], f32, tag="RN")
                np_, nn_ = len(pos_strips), len(neg_strips)

                def rowsum(dst, t, n):
                    if n == 0:
                        nc.vector.memset(dst, 0.0)
                        return
                    v = t[:, 0 : n * C].rearrange("p (r c) -> p c r", r=n, c=C)
                    nc.vector.tensor_reduce(
                        dst, v, axis=mybir.AxisListType.X, op=AL.add
                    )

                rowsum(RP[:], POS, np_)
                rowsum(RN[:], NEG, nn_)
                nc.vector.tensor_sub(res[:], RP[:], RN[:])
                if it == repeat - 1:
                    nc.sync.dma_start(out=out_p[:], in_=res[:])

    return nc


def _pack_inputs_fold(x, p_w, np_dt, f_inner=False):
    """Per-core [128, C*FE] blocks of x scaled by sqrt(.5|pw|), e-sorted.

    f_inner: lay out as [batch, e, f] so sign groups are contiguous."""
    pw = p_w.reshape(E).astype(np.float64)
    perm = np.argsort(pw < 0, kind="stable")       # pw>=0 first
    e_plus = int((pw >= 0).sum())
    w = np.sqrt(0.5 * np.abs(pw[perm]))
    xs = x.reshape(B, F, E)[:, :, perm] * w[None, None, :]
    if f_inner:
        xs = np.swapaxes(xs, 1, 2)                 # [B, E, F]
    xs = xs.reshape(B, FE).astype(np_dt)
    blocks = []
    for i in range(N_CORES):
        sh = xs[i * B_LOC : (i + 1) * B_LOC]
        xb = np.concatenate(
            [sh[c * 128 : (c + 1) * 128] for c in range(C)], axis=1
        )
        blocks.append(np.ascontiguousarray(xb))
    return blocks, e_plus


def _get_nc(dt_x_name: str = "float16"):
    key = (dt_x_name, USE_ACT)
    if key not in _BUILD_CACHE:
        _BUILD_CACHE[key] = _build(dt_x_name, use_act=USE_ACT)
    return _BUILD_CACHE[key]


def _pack_inputs(x, p_w, np_dt):
    """Per-core [128, XCOLS] blocks: row p = [x[b=p] | x[b=128+p] | 0.5*p_w tiled]."""
    xs = x.reshape(B, FE).astype(np_dt)
    w = np.tile((0.5 * p_w.reshape(E)).astype(np_dt), C)      # [C*E]
    pw_block = np.broadcast_to(w, (128, C * E))
    blocks = []
    for i in range(N_CORES):
        sh = xs[i * B_LOC : (i + 1) * B_LOC]                  # [256, FE]
        xb = np.concatenate(
            [sh[c * 128 : (c + 1) * 128] for c in range(C)] + [pw_block], axis=1
        )
        blocks.append(np.ascontiguousarray(xb))
    return blocks


VARIANT = "fold"


def kernel(x, W_w, W_b, h_w, h_b, p_w, _trace: bool = False, _dt: str = "float16"):
    from concourse.bass_utils import run_bass_kernel_spmd

    x = np.asarray(x, dtype=np.float32)
    p_w = np.asarray(p_w, dtype=np.float32)

    np_dt = {"float16": np.float16, "float32": np.float32}[_dt]
    if VARIANT == "fold":
        blocks, e_plus = _pack_inputs_fold(x, p_w, np_dt)
        key = ("fold", _dt, e_plus)
        if key not in _BUILD_CACHE:
            _BUILD_CACHE[key] = _build_fold(e_plus, _dt, act_cols=28)
        nc = _BUILD_CACHE[key]
    else:
        blocks = _pack_inputs(x, p_w, np_dt)
        nc = _get_nc(_dt)
    in_maps = [{"xin": blocks[i]} for i in range(N_CORES)]
    r = run_bass_kernel_spmd(nc, in_maps, list(range(N_CORES)), trace=_trace)

    out = np.empty((B,), dtype=np.float32)
    for i in range(N_CORES):
        res = np.asarray(r.results[i]["out"])                 # [128, C]
        out[i * B_LOC : (i + 1) * B_LOC] = res.T.reshape(-1)
    if _trace:
        kernel.last_result = r
    return out.reshape(B, 1)
